# revision 3
# baseline (speedup 1.0000x reference)
"""LoRA layer kernel for Trainium2: out = (W + A@B) @ x.

Shapes (fp32): W [4096,4096], A [4096,16], B [16,4096], x [4096,8192],
out [4096,8192].

Strategy (2-way token x 4-way row tensor-parallel, 8 NeuronCores):
The workload is HBM-bandwidth-bound when x is replicated to all 8 cores
(row-only sharding makes the chip read x 8 times: 537 MB/pass vs a
~1 TB/s shared HBM ceiling). So shard 2D instead:
- cores c = nh*4 + mq own token-half nh (4096 cols of x) and row-quarter
  mq (1024 rows of W/A). Chip-wide x traffic drops to 4 copies (268 MB),
  out is written as fp16 (67 MB) and upcast on the host.
- Per core, on device:
    1. Load W_quarter^T (pre-cast fp16 on host, 8.4 MB) into SBUF in 8
       ko-group chunks on the gpsimd DMA ring.
    2. delta^T = B^T @ A_q^T via 64 K=16 fp16 matmuls (fp32 PSUM); add in
       place into the fp16 W^T tiles (VectorE) giving W'^T (resident).
    3. Stream this core's x half (pre-cast fp16) in 8 n-tiles of 512
       columns, alternating the two HWDGE DMA rings (sync + scalar),
       3-deep prefetch; per n-tile compute 8 (m) x 32 (k) fp16 matmuls
       accumulating fp32 in PSUM; evict to SBUF as fp16 (VectorE) and DMA
       out on the gpsimd ring.
- Host gathers the 4x2 grid of [1024, 4096] fp16 shards, upcasts fp32.

fp16 keeps the norm-relative error at ~4e-4 (hardware-measured): the PE
runs 16-bit matmuls 4x faster than fp32 with fp32 PSUM accumulation, and
the fp16 output store adds only ~2e-4.
"""

import numpy as np

import concourse.bacc as bacc
import concourse.mybir as mybir
import concourse.tile as tile
from concourse.bass_utils import run_bass_kernel_spmd

P = 128          # partitions / systolic dim
OUT = 4096
IN = 4096
RANK = 16
NTOK = 8192
NCORES = 8
NSH = 2          # token shards
MSHARDS = 4      # row shards
MSH = OUT // MSHARDS         # 1024 output rows per core
TOK = NTOK // NSH            # 4096 tokens per core
KS = IN // P                 # 32 k-subtiles
MO = MSH // P                # 8 m-subtiles per core
NT = TOK // 512              # 8 n-tiles per core
NF = 512                     # columns per n-tile
NG = 8                       # W ko-group chunks
KG = KS // NG                # 4 ko per group
XBUFS = 3                    # x-tile prefetch depth


def _build(reps=None):
    """Build the per-core Bass program. reps=None -> single pass (graded
    kernel); reps=k -> main loop wrapped in a hardware For_i(0, k) for
    dispatch-floor-free timing (used by test.py)."""
    nc = bacc.Bacc(None, target_bir_lowering=False, debug=False)

    wT = nc.dram_tensor("wT", [NG, P, KG, MSH], mybir.dt.float16, kind="ExternalInput")
    aT = nc.dram_tensor("aT", [RANK, MSH], mybir.dt.float16, kind="ExternalInput")
    b = nc.dram_tensor("b", [RANK, KS, P], mybir.dt.float16, kind="ExternalInput")
    xh = nc.dram_tensor("xh", [NT, P, KS, NF], mybir.dt.float16, kind="ExternalInput")
    out = nc.dram_tensor("out", [NT, P, MO, NF], mybir.dt.float16, kind="ExternalOutput")

    with tile.TileContext(nc) as tc:
        with (
            tc.tile_pool(name="w16pool", bufs=1) as w16pool,
            tc.tile_pool(name="spool", bufs=1) as spool,
            tc.tile_pool(name="xpool", bufs=XBUFS) as xpool,
            tc.tile_pool(name="opool", bufs=2) as opool,
            tc.tile_pool(name="psum", bufs=8, space="PSUM") as psum,
        ):
            aT_sb = spool.tile([RANK, MSH], mybir.dt.float16)
            b_sb = spool.tile([RANK, KS, P], mybir.dt.float16)
            nc.gpsimd.dma_start(aT_sb[:], aT.ap())
            nc.gpsimd.dma_start(b_sb[:], b.ap())

            wp16_g = []
            for g in range(NG):
                wp = w16pool.tile([P, KG, MSH], mybir.dt.float16,
                                  tag=f"wp{g}", bufs=1, name=f"wp{g}")
                nc.gpsimd.dma_start(wp[:], wT.ap()[g])
                wp16_g.append(wp)

            # W'^T = W^T + (A@B)^T, rounded to fp16; MSH=1024 -> two 512-wide
            # PSUM slices per ko.
            # Per ko: psum[p, m] = sum_r b[r, ko*128+p] * aT[r, m]  (K=16)
            for ko in range(KS):
                g, kg = divmod(ko, KG)
                for mh in range(MSH // NF):
                    ms = slice(mh * NF, (mh + 1) * NF)
                    dps = psum.tile([P, NF], mybir.dt.float32, tag="ps", name="dps")
                    nc.tensor.matmul(dps[:], b_sb[:, ko], aT_sb[:, ms],
                                     start=True, stop=True)
                    nc.vector.tensor_add(wp16_g[g][:, kg, ms],
                                         wp16_g[g][:, kg, ms], dps[:])

            # Main: out[m, n] = sum_k W'[m, k] x[k, n], fp16 in, fp32 accum,
            # fp16 out.
            def body():
                for nt in range(NT):
                    xt = xpool.tile([P, KS, NF], mybir.dt.float16, tag="xt", name="xt")
                    ring = nc.sync if nt % 2 == 0 else nc.scalar
                    ring.dma_start(xt[:], xh.ap()[nt])
                    ot = opool.tile([P, MO, NF], mybir.dt.float16, tag="ot", name="ot")
                    for mo in range(MO):
                        mps = psum.tile([P, NF], mybir.dt.float32, tag="ps", name="mps")
                        for ko in range(KS):
                            g, kg = divmod(ko, KG)
                            nc.tensor.matmul(
                                mps[:],
                                wp16_g[g][:, kg, mo * P:(mo + 1) * P],
                                xt[:, ko],
                                start=(ko == 0),
                                stop=(ko == KS - 1),
                            )
                        nc.vector.tensor_copy(ot[:, mo], mps[:])
                    nc.gpsimd.dma_start(out.ap()[nt], ot[:])

            if reps is None:
                body()
            else:
                with tc.For_i(0, reps):
                    body()

    nc.compile()
    return nc


def _build_nc():
    return _build(None)


_NC_CACHE = None


def _get_nc():
    global _NC_CACHE
    if _NC_CACHE is None:
        _NC_CACHE = _build_nc()
    return _NC_CACHE


def prepare_in_maps(x, weight, A, B):
    """Shard + lay out the full inputs into per-core device input maps."""
    x = np.ascontiguousarray(x, dtype=np.float32)
    weight = np.ascontiguousarray(weight, dtype=np.float32)
    A = np.ascontiguousarray(A, dtype=np.float32)
    B = np.ascontiguousarray(B, dtype=np.float32)

    # x [IN, NTOK] fp16; token half nh -> [nt, p, ko, j]
    # with k = ko*128+p, global col = nh*TOK + nt*512 + j
    x16 = x.astype(np.float16).reshape(KS, P, NSH, NT, NF)
    xh_half = [np.ascontiguousarray(x16[:, :, nh].transpose(2, 1, 0, 3))
               for nh in range(NSH)]
    # B [RANK, IN] -> [r, ko, p]
    b_dev = np.ascontiguousarray(B.astype(np.float16).reshape(RANK, KS, P))

    in_maps = []
    for c in range(NCORES):
        nh, mq = divmod(c, MSHARDS)
        rows = slice(mq * MSH, (mq + 1) * MSH)
        # W_quarter^T [k, m] -> [g, p, kg, m] with k = (g*KG + kg)*P + p
        wT_dev = np.ascontiguousarray(
            weight[rows].T.astype(np.float16).reshape(NG, KG, P, MSH).transpose(0, 2, 1, 3)
        )
        aT_dev = np.ascontiguousarray(A[rows].T.astype(np.float16))
        in_maps.append({"wT": wT_dev, "aT": aT_dev, "b": b_dev, "xh": xh_half[nh]})
    return in_maps


def assemble_output(results):
    """Gather per-core fp16 [nt, p, mo, j] outputs into [OUT, NTOK] fp32."""
    out = np.empty((OUT, NTOK), dtype=np.float32)
    for c, r in enumerate(results):
        nh, mq = divmod(c, MSHARDS)
        shard = r["out"].transpose(2, 1, 0, 3).reshape(MSH, TOK).astype(np.float32)
        out[mq * MSH:(mq + 1) * MSH, nh * TOK:(nh + 1) * TOK] = shard
    return out


def kernel(x, weight, A, B):
    nc = _get_nc()
    in_maps = prepare_in_maps(x, weight, A, B)
    res = run_bass_kernel_spmd(nc, in_maps, core_ids=list(range(NCORES)))
    return assemble_output(res.results)


if __name__ == "__main__":
    rng = np.random.default_rng(0)
    x = rng.standard_normal((IN, NTOK), dtype=np.float32)
    weight = rng.standard_normal((OUT, IN), dtype=np.float32)
    A = rng.standard_normal((OUT, RANK), dtype=np.float32)
    B = rng.standard_normal((RANK, IN), dtype=np.float32)
    got = kernel(x, weight, A, B)
    ref = (weight.astype(np.float64) + A.astype(np.float64) @ B.astype(np.float64)) @ x.astype(np.float64)
    err = np.abs(got - ref).max() / np.abs(ref).max()
    rel = np.linalg.norm(got - ref) / np.linalg.norm(ref)
    print("max-rel-to-max err:", err, " norm-rel:", rel)
